# revision 7
# baseline (speedup 1.0000x reference)
"""PointCrop2D on 8 Trainium2 NeuronCores.

Per sample: x = clip(int(points[s,0,0]), 1, 510), y = clip(int(points[s,1,0]), 1, 510);
output[s] = 224x224x3 crop of image rows y-112..y+111, cols x-112..x+111, with
out-of-range rows/cols (and image row/col 511) replaced by fill = -2.0.

Strategy: batch-shard 4 samples/core. Host builds a column-padded flat image
buffer (W 512 -> 736, image col 511 and out-of-range cols pre-filled) plus a
dedicated fill row, and computes one int32 flat gather index per (sample, crop
row) -- invalid rows point at the fill row. Device does one big indirect DMA
gather (896 rows x 2688B) into SBUF and one store to the output DRAM tensor.
Host reassembles the 8 per-core outputs.
"""

import sys

sys.path.insert(0, "/opt/trn_rl_repo")

import numpy as np

B, H, W, C = 32, 512, 512, 3
CROP = 224
DIA = CROP // 2  # 112
FILL = np.float32((0.0 - 0.45) / 0.225)  # -2.0
NCORES = 8
BS = B // NCORES  # 4 samples per core
WPAD = W + 2 * DIA  # 736
ROWE = WPAD * C  # 2208 elems per padded row
SAMPE = H * ROWE  # elems per padded sample
IMGN = BS * SAMPE + ROWE  # flat per-core buffer + trailing fill row
IMGROWS = BS * H + 1  # 2049 padded rows incl. trailing fill row
NIDX = 2 * BS  # 8 gather columns: (sample, half)
ROWB = CROP * C  # 672 elems per crop row

# knobs read by test.py
TRACE = False
LAST = None  # BassKernelResults of the most recent run

_NC_CACHE = {}


def _build_nc(chunks: int):
    from contextlib import ExitStack

    from concourse import bass, mybir

    f32 = mybir.dt.float32
    i32 = mybir.dt.int32

    nc = bass.Bass(target_bir_lowering=False)
    img = nc.dram_tensor("img", [IMGROWS, ROWE], f32, kind="ExternalInput")
    idx = nc.dram_tensor("idx", [DIA, NIDX], i32, kind="ExternalInput")
    out = nc.dram_tensor("out", [DIA, NIDX * ROWB], f32, kind="ExternalOutput")

    assert NIDX % chunks == 0
    cw = NIDX // chunks  # gather columns per store chunk

    # Raw Bass (no TileContext): Tile's exit drain waits on every DMA
    # semaphore at once, which overflows walrus' per-instruction sync-wait
    # budget. Manual sems keep every wait a single-sem sequencer wait.
    #
    # The HW DGE consumes exactly one offset per partition of the gather
    # output (sim generalizes to many — don't trust it here), so each
    # indirect DMA uses a single idx column [DIA, 1] -> out [DIA, ROWB].
    with ExitStack() as es:
        block = es.enter_context(nc.Block())
        s_idx = es.enter_context(nc.semaphore("s_idx"))
        g_sems = [es.enter_context(nc.semaphore(f"s_g{c}")) for c in range(chunks)]
        st_sems = [es.enter_context(nc.semaphore(f"s_st{c}")) for c in range(chunks)]
        idx_t = es.enter_context(nc.sbuf_tensor("idx_t", [DIA, NIDX], i32))
        g = es.enter_context(nc.sbuf_tensor("g", [DIA, NIDX * ROWB], f32))

        @block.gpsimd
        def _(gpsimd):
            gpsimd.dma_start(idx_t[:], idx[:]).then_inc(s_idx, 16)
            gpsimd.wait_ge(s_idx, 16)
            for k in range(NIDX):
                gpsimd.indirect_dma_start(
                    out=g[:, k * ROWB : (k + 1) * ROWB],
                    out_offset=None,
                    in_=img[:],
                    in_offset=bass.IndirectOffsetOnAxis(
                        ap=idx_t[:, k : k + 1], axis=1
                    ),
                ).then_inc(g_sems[k // cw], 16)

        @block.sync
        def _(sync):
            for c in range(chunks):
                sync.wait_ge(g_sems[c], 16 * cw)
                sync.dma_start(
                    out[:, c * cw * ROWB : (c + 1) * cw * ROWB],
                    g[:, c * cw * ROWB : (c + 1) * cw * ROWB],
                ).then_inc(st_sems[c], 16)
            for c in range(chunks):
                sync.wait_ge(st_sems[c], 16)

    return nc


def kernel(points, images, chunks: int = 2):
    global LAST
    from concourse import bass_utils

    points = np.asarray(points)
    images = np.asarray(images, dtype=np.float32)

    x = np.clip(points[:, 0, 0].astype(np.int32), 1, W - 2)  # (B,)
    y = np.clip(points[:, 1, 0].astype(np.int32), 1, H - 2)

    # column-padded per-core flat buffers; image col 511 left as fill
    flat = np.full((NCORES, IMGN), FILL, np.float32)
    buf = flat[:, : BS * SAMPE].reshape(NCORES, BS, H, WPAD, C)
    buf[:, :, :, DIA : DIA + W - 1, :] = images.reshape(NCORES, BS, H, W, C)[
        :, :, :, : W - 1, :
    ]

    # flat gather index per (sample, crop row); invalid rows -> fill row
    r = y[:, None] - DIA + np.arange(CROP)[None, :]  # (B, 224)
    valid = (r >= 0) & (r <= H - 2)
    s_local = np.arange(B) % BS
    base = s_local[:, None] * SAMPE + r * ROWE + (x * C)[:, None]
    idxf = np.where(valid, base, BS * SAMPE).astype(np.int32)  # (B, 224)
    idx_arr = np.ascontiguousarray(
        idxf.reshape(NCORES, BS, 2, DIA).transpose(0, 3, 1, 2).reshape(NCORES, DIA, NIDX)
    )

    key = chunks
    if key not in _NC_CACHE:
        _NC_CACHE[key] = _build_nc(chunks)
    nc = _NC_CACHE[key]

    in_maps = [
        {"img": flat[c].reshape(IMGROWS, ROWE), "idx": idx_arr[c]}
        for c in range(NCORES)
    ]
    LAST = bass_utils.run_bass_kernel_spmd(
        nc, in_maps, list(range(NCORES)), trace=TRACE
    )

    outs = np.stack([LAST.results[c]["out"] for c in range(NCORES)])  # (8,112,5376)
    return np.ascontiguousarray(
        outs.reshape(NCORES, DIA, BS, 2, ROWB)
        .transpose(0, 2, 3, 1, 4)
        .reshape(B, CROP, CROP, C)
    )
